# revision 2
# baseline (speedup 1.0000x reference)
"""ButterflyLinear Trainium2 kernel.

Math: out[b, s, i] = (sum_o x[b, s, o] * W[o, i]) * mask[s, i], with
mask[s, i] = 1 iff 4s <= i < 4s+4 (stride-4 band). The band makes the
output block-diagonal: s-block t (128 rows) only touches output columns
[512t, 512t+512). So instead of the full (16*1024, 1024) @ (1024, 4096)
matmul we compute 8 diagonal blocks per batch: x[b, 128t:128t+128, :] @
W[:, 512t:512t+512] -- an 8x compute reduction.

Sharding (8 cores): core t owns s-block t for all 16 batches
(tensor-parallel split of W columns; no inter-core communication).

Per-core device program:
  - W block (1024, 512) resident in SBUF as 8 chunks of (128 o, 512 i)
  - x slice pre-transposed on host to (o, b, s) so the contraction dim o
    sits on SBUF partitions (PE matmul contracts over partitions)
  - 16 accumulation groups (one per batch): psum[128 s, 512 i] +=
    xT_chunk.T @ W_chunk over 8 o-chunks
  - band mask applied on DVE while evacuating PSUM, then DMA out
Host assembles the 8 (16, 128, 512) block outputs into the zero-filled
(16, 1024, 4096) result.
"""

import sys
from contextlib import ExitStack

import numpy as np

if "/opt/trn_rl_repo" not in sys.path:
    sys.path.insert(0, "/opt/trn_rl_repo")

import concourse.bass as bass  # noqa: E402
import concourse.tile as tile  # noqa: E402
from concourse import bacc, mybir  # noqa: E402
from concourse.bass_utils import run_bass_kernel_spmd  # noqa: E402

B = 16  # batch
NT = 8  # s-blocks == cores
SB = 128  # s rows per block
NC_ = 8  # o chunks
KC = 128  # o rows per chunk
NI = 512  # output columns per block

# fp32r streams the PE at 1 row/cycle (vs 4 for fp32) at reduced multiply
# precision; flip to float32 if the accuracy check needs it.
MM_DT = mybir.dt.float32r
F32 = mybir.dt.float32

_STATE: dict = {}


def _build():
    if "nc" in _STATE:
        return _STATE["nc"]

    nc = bacc.Bacc(
        "TRN2", target_bir_lowering=False, debug=False, num_devices=NT
    )
    xt = nc.dram_tensor("xt", [NC_, KC, B, SB], MM_DT, kind="ExternalInput").ap()
    wt = nc.dram_tensor("wt", [NC_, KC, NI], MM_DT, kind="ExternalInput").ap()
    mask = nc.dram_tensor("mask", [SB, NI], F32, kind="ExternalInput").ap()
    out = nc.dram_tensor("out", [B, SB, NI], F32, kind="ExternalOutput").ap()

    with tile.TileContext(nc) as tc, ExitStack() as ctx:
        wp = ctx.enter_context(tc.tile_pool(name="w", bufs=1))
        xp = ctx.enter_context(tc.tile_pool(name="x", bufs=1))
        mp = ctx.enter_context(tc.tile_pool(name="m", bufs=1))
        pp = ctx.enter_context(tc.tile_pool(name="ps", bufs=8, space="PSUM"))
        op = ctx.enter_context(tc.tile_pool(name="o", bufs=4))

        mask_t = mp.tile([SB, NI], F32, tag="mask")
        nc.sync.dma_start(out=mask_t[:], in_=mask[:])

        w_t = []
        x_t = []
        for c in range(NC_):
            w = wp.tile([KC, NI], MM_DT, tag=f"w{c}")
            nc.sync.dma_start(out=w[:], in_=wt[c])
            w_t.append(w)
            xc = xp.tile([KC, B, SB], MM_DT, tag=f"x{c}")
            nc.sync.dma_start(out=xc[:], in_=xt[c])
            x_t.append(xc)

        for g in range(2):
            ps = [pp.tile([SB, NI], F32, tag="ps", name=f"ps_{g}_{i}") for i in range(8)]
            for c in range(NC_):
                for bi in range(8):
                    b = g * 8 + bi
                    nc.tensor.matmul(
                        ps[bi][:],
                        x_t[c][:, b, :],
                        w_t[c][:],
                        start=(c == 0),
                        stop=(c == NC_ - 1),
                    )
            for bi in range(8):
                b = g * 8 + bi
                ot = op.tile([SB, NI], F32, tag="ot")
                nc.vector.tensor_mul(ot[:], ps[bi][:], mask_t[:])
                nc.sync.dma_start(out=out[b], in_=ot[:])

    nc.compile()
    _STATE["nc"] = nc
    return nc


def _shard(x, W):
    x = np.ascontiguousarray(np.asarray(x, dtype=np.float32))
    W = np.ascontiguousarray(np.asarray(W, dtype=np.float32))
    # xt[t][c, p, b, s] = x[b, 128t+s, 128c+p]
    xr = x.reshape(B, NT, SB, NC_, KC)  # [b, t, s, c, p]
    xts = np.ascontiguousarray(np.transpose(xr, (1, 3, 4, 0, 2)))
    # wt[t][c, p, i] = W[128c+p, 512t+i]
    wr = W.reshape(NC_, KC, NT, NI)  # [c, p, t, i]
    wts = np.ascontiguousarray(np.transpose(wr, (2, 0, 1, 3)))
    m = np.zeros((SB, NI), dtype=np.float32)
    r = np.arange(SB)
    for j in range(4):
        m[r, 4 * r + j] = 1.0
    return [
        {"xt": xts[t], "wt": wts[t], "mask": m} for t in range(NT)
    ]


def kernel(x, W, _trace=False, _trace_kwargs=None):
    nc = _build()
    in_maps = _shard(x, W)
    res = run_bass_kernel_spmd(
        nc,
        in_maps,
        list(range(NT)),
        trace=_trace,
        **(_trace_kwargs or {}),
    )
    _STATE["last_run"] = res
    y = np.zeros((B, NT * SB, NT * NI), dtype=np.float32)
    yv = y.reshape(B, NT, SB, NT, NI)
    for t in range(NT):
        yv[:, t, :, t, :] = res.results[t]["out"]
    return y


# revision 6
# speedup vs baseline: 1.0339x; 1.0339x over previous
"""ButterflyLinear Trainium2 kernel.

Math: out[b, s, i] = (sum_o x[b, s, o] * W[o, i]) * mask[s, i], with
mask[s, i] = 1 iff 4s <= i < 4s+4 (stride-4 band). The band makes the
output block-diagonal: s-block t (128 rows) only touches output columns
[512t, 512t+512). So instead of the full (16*1024, 1024) @ (1024, 4096)
matmul we compute 8 diagonal blocks per batch: x[b, 128t:128t+128, :] @
W[:, 512t:512t+512] -- an 8x compute reduction.

Sharding (8 cores): core t owns s-block t for all 16 batches
(tensor-parallel split of W columns; no inter-core communication).

Per-core device program:
  - W block (1024, 512) resident in SBUF as 8 chunks of (128 o, 512 i)
  - x slice pre-transposed on host to (o, b, s) so the contraction dim o
    sits on SBUF partitions (PE matmul contracts over partitions)
  - dummy warm-up matmuls during the DMA ramp so the HAM clock gate is
    released (2.4 GHz) before the real stream starts
  - 16 accumulation chains (one per batch, two waves of 8 PSUM banks):
    psum[128 s, 512 i] += xT_chunk.T @ W_chunk over 8 o-chunks (fp32r)
  - PSUM evacuated with plain copies alternating ScalarE/VectorE, raw
    blocks DMA'd out (no masking on device)
Host extracts the 4-wide diagonal band from each raw block into the
zero-filled (16, 1024, 4096) result; off-band values are simply ignored.
"""

import sys
from contextlib import ExitStack

import numpy as np

if "/opt/trn_rl_repo" not in sys.path:
    sys.path.insert(0, "/opt/trn_rl_repo")

import concourse.bass as bass  # noqa: E402
import concourse.tile as tile  # noqa: E402
from concourse import bacc, mybir  # noqa: E402
from concourse.bass_utils import run_bass_kernel_spmd  # noqa: E402

import os

B = 16  # batch
NT = 8  # s-blocks == cores
SB = 128  # s rows per block
NC_ = 8  # o chunks
KC = 128  # o rows per chunk
NI = 512  # output columns per block
N_WARMUP = int(os.environ.get("BFK_WARMUP", "14"))  # HAM warm-up matmuls
ACT_COPY = os.environ.get("BFK_ACT_COPY", "1") == "1"

# fp32r streams the PE at 1 row/cycle (vs 4 for fp32) at ~tf32 precision
# (measured band rel err 1.4e-4); flip to float32 if accuracy requires.
MM_DT = mybir.dt.float32r
F32 = mybir.dt.float32

_STATE: dict = {}


def _build():
    if "nc" in _STATE:
        return _STATE["nc"]

    nc = bacc.Bacc(
        "TRN2", target_bir_lowering=False, debug=False, num_devices=NT
    )
    xt = nc.dram_tensor("xt", [NC_, KC, B, SB], MM_DT, kind="ExternalInput").ap()
    wt = nc.dram_tensor("wt", [NC_, KC, NI], MM_DT, kind="ExternalInput").ap()
    out = nc.dram_tensor("out", [B, SB, NI], F32, kind="ExternalOutput").ap()

    with tile.TileContext(nc) as tc, ExitStack() as ctx:
        wp = ctx.enter_context(tc.tile_pool(name="w", bufs=1))
        xp = ctx.enter_context(tc.tile_pool(name="x", bufs=1))
        pp = ctx.enter_context(tc.tile_pool(name="ps", bufs=8, space="PSUM"))
        op = ctx.enter_context(tc.tile_pool(name="o", bufs=6))
        sp = ctx.enter_context(tc.tile_pool(name="scratch", bufs=1))

        # PE warm-up: garbage matmuls on a memset scratch tile, no DMA
        # deps, so they run during the input-DMA ramp and release the HAM
        # clock gate (~3.4us of sustained PE activity) before the real
        # matmuls arrive.
        if N_WARMUP:
            # Plain f32 (2-pass matmuls, ~4 cycles/row) so each warm-up op
            # covers more wall time; f32r scratch matmuls were observed to
            # wedge the exec unit.
            wmt = sp.tile([KC, NI], F32, tag="warm")
            nc.gpsimd.memset(wmt[:], 0.0)
            pwarm = pp.tile([SB, NI], F32, tag="ps", name="ps_warm")
            for _ in range(N_WARMUP):
                nc.tensor.matmul(
                    pwarm[:], wmt[:, :KC], wmt[:], start=True, stop=True
                )

        w_t = []
        x_t = []
        for c in range(NC_):
            w = wp.tile([KC, NI], MM_DT, tag=f"w{c}")
            nc.sync.dma_start(out=w[:], in_=wt[c])
            w_t.append(w)
            xc = xp.tile([KC, B, SB], MM_DT, tag=f"x{c}")
            nc.sync.dma_start(out=xc[:], in_=xt[c])
            x_t.append(xc)

        for g in range(2):
            ps = [
                pp.tile([SB, NI], F32, tag="ps", name=f"ps_{g}_{i}")
                for i in range(8)
            ]
            for c in range(NC_):
                for bi in range(8):
                    b = g * 8 + bi
                    nc.tensor.matmul(
                        ps[bi][:],
                        x_t[c][:, b, :],
                        w_t[c][:],
                        start=(c == 0),
                        stop=(c == NC_ - 1),
                    )
            for bi in range(8):
                b = g * 8 + bi
                ot = op.tile([SB, NI], F32, tag="ot")
                # Alternate evacuation between VectorE and ScalarE so the
                # copies drain two banks at a time.
                if ACT_COPY and bi % 2 == 1:
                    nc.scalar.copy(ot[:], ps[bi][:])
                else:
                    nc.vector.tensor_copy(ot[:], ps[bi][:])
                nc.sync.dma_start(out=out[b], in_=ot[:])

    nc.compile()
    _STATE["nc"] = nc
    return nc


def _shard(x, W):
    x = np.ascontiguousarray(np.asarray(x, dtype=np.float32))
    W = np.ascontiguousarray(np.asarray(W, dtype=np.float32))
    # xt[t][c, p, b, s] = x[b, 128t+s, 128c+p]
    xr = x.reshape(B, NT, SB, NC_, KC)  # [b, t, s, c, p]
    xts = np.ascontiguousarray(np.transpose(xr, (1, 3, 4, 0, 2)))
    # wt[t][c, p, i] = W[128c+p, 512t+i]
    wr = W.reshape(NC_, KC, NT, NI)  # [c, p, t, i]
    wts = np.ascontiguousarray(np.transpose(wr, (2, 0, 1, 3)))
    return [{"xt": xts[t], "wt": wts[t]} for t in range(NT)]


def kernel(x, W, _trace=False, _trace_kwargs=None):
    nc = _build()
    in_maps = _shard(x, W)
    res = run_bass_kernel_spmd(
        nc,
        in_maps,
        list(range(NT)),
        trace=_trace,
        **(_trace_kwargs or {}),
    )
    _STATE["last_run"] = res
    y = np.zeros((B, NT * SB, NT * NI), dtype=np.float32)
    # Band extraction: y[b, s, 4s+j] = block_t[b, r, 4r+j], s = 128t + r.
    band = np.empty((B, NT * SB, 4), dtype=np.float32)
    for t in range(NT):
        blk = np.ascontiguousarray(res.results[t]["out"])  # (B, 128, 512)
        st = blk.strides
        diag = np.lib.stride_tricks.as_strided(
            blk, shape=(B, SB, 4), strides=(st[0], st[1] + 4 * st[2], st[2])
        )
        band[:, t * SB : (t + 1) * SB, :] = diag
    s_idx = np.arange(NT * SB)
    y4 = y.reshape(B, NT * SB, NT * SB, 4)
    y4[:, s_idx, s_idx, :] = band
    return y


# revision 7
# speedup vs baseline: 1.3222x; 1.2788x over previous
"""ButterflyLinear Trainium2 kernel.

Math: out[b, s, i] = (sum_o x[b, s, o] * W[o, i]) * mask[s, i], with
mask[s, i] = 1 iff 4s <= i < 4s+4 (stride-4 band). The band makes the
output block-diagonal: s-rows [128t, 128t+128) only touch output columns
[512t, 512t+512) -- an 8x compute reduction vs the full matmul.

Sharding (8 cores): core t owns s-block t for all 16 batches
(tensor-parallel split of W columns; no inter-core communication).

Key packing trick: a 64-row s-sub-block only spans a 256-wide band
window, and that window is the same for every batch. So the matmul
stationary packs TWO batches on the partition axis (M = 128 = 2 batches
x 64 s-rows) against one N=256 W window -- W streams once per batch
PAIR, halving PE row traffic, and each accumulation lives in HALF a
PSUM bank. All 16 chains (8 batch-pairs x 2 sub-blocks) fit in the 8
banks at once: a single wave, no bank-recycling serialization.
PSUM has_written is per-element: only the first matmul of each bank
uses start=True (clears the whole bank), its half-bank partner starts
with start=False and overwrites its untouched half.

Per-core device program:
  - f32 warm-up matmuls during the DMA ramp (HAM clock-gate release)
  - 8 o-chunks streamed (W chunk 256KB + x chunk 1MB each), fp32r
  - 16 chains x 8 chunk-matmuls (N=256) accumulating in half-banks
  - 8 full-bank copies (alternating VectorE/ScalarE) -> DMA raw blocks
Host extracts the 4-wide diagonal band from the raw blocks into the
zero-filled (16, 1024, 4096) result.
"""

import os
import sys
from contextlib import ExitStack

import numpy as np

if "/opt/trn_rl_repo" not in sys.path:
    sys.path.insert(0, "/opt/trn_rl_repo")

import concourse.bass as bass  # noqa: E402
import concourse.tile as tile  # noqa: E402
from concourse import bacc, mybir  # noqa: E402
from concourse.bass_utils import run_bass_kernel_spmd  # noqa: E402

B = 16  # batch
NT = 8  # s-blocks == cores
SB = 128  # s rows per block
NC_ = 8  # o chunks
KC = 128  # o rows per chunk
NI = 512  # output columns per block
NG = 8  # batch pairs
NH = 2  # 64-row s-sub-blocks per s-block
NW = 256  # W window per sub-block
N_WARMUP = int(os.environ.get("BFK_WARMUP", "6"))  # HAM warm-up matmuls

# fp32r streams the PE at ~2x the fp32 rate at ~tf32 precision
# (measured band rel err 1.4e-4); flip to float32 if accuracy requires.
MM_DT = mybir.dt.float32r
F32 = mybir.dt.float32

_STATE: dict = {}


def _build():
    if "nc" in _STATE:
        return _STATE["nc"]

    nc = bacc.Bacc(
        "TRN2", target_bir_lowering=False, debug=False, num_devices=NT
    )
    xt = nc.dram_tensor(
        "xt", [NC_, KC, NG, NH, SB], MM_DT, kind="ExternalInput"
    ).ap()
    wt = nc.dram_tensor("wt", [NC_, KC, NH, NW], MM_DT, kind="ExternalInput").ap()
    out = nc.dram_tensor("out", [NG, SB, NI], F32, kind="ExternalOutput").ap()

    with tile.TileContext(nc) as tc, ExitStack() as ctx:
        wp = ctx.enter_context(tc.tile_pool(name="w", bufs=1))
        xp = ctx.enter_context(tc.tile_pool(name="x", bufs=1))
        pp = ctx.enter_context(tc.tile_pool(name="ps", bufs=8, space="PSUM"))
        op = ctx.enter_context(tc.tile_pool(name="o", bufs=6))
        sp = ctx.enter_context(tc.tile_pool(name="scratch", bufs=1))

        # PE warm-up: f32 matmuls (2 HW passes each) on a memset scratch
        # tile, no DMA deps, so they run during the input-DMA ramp and
        # release the HAM clock gate before the real stream starts.
        if N_WARMUP:
            wmt = sp.tile([KC, NI], F32, tag="warm")
            nc.gpsimd.memset(wmt[:], 0.0)
            pwarm = pp.tile([SB, NI], F32, tag="ps", name="ps_warm")
            for _ in range(N_WARMUP):
                nc.tensor.matmul(
                    pwarm[:], wmt[:, :KC], wmt[:], start=True, stop=True
                )

        w_t = []
        x_t = []
        for c in range(NC_):
            w = wp.tile([KC, NH, NW], MM_DT, tag=f"w{c}")
            nc.sync.dma_start(out=w[:], in_=wt[c])
            w_t.append(w)
            xc = xp.tile([KC, NG, NH, SB], MM_DT, tag=f"x{c}")
            nc.sync.dma_start(out=xc[:], in_=xt[c])
            x_t.append(xc)

        ps = [pp.tile([SB, NI], F32, tag="ps", name=f"ps_{g}") for g in range(NG)]
        for c in range(NC_):
            for g in range(NG):
                for h in range(NH):
                    nc.tensor.matmul(
                        ps[g][:, h * NW : (h + 1) * NW],
                        x_t[c][:, g, h, :],
                        w_t[c][:, h, :],
                        start=(c == 0 and h == 0),
                        stop=(c == NC_ - 1 and h == NH - 1),
                    )
        for g in range(NG):
            ot = op.tile([SB, NI], F32, tag="ot")
            # Alternate evacuation between VectorE and ScalarE so two
            # banks drain at a time.
            if g % 2 == 1:
                nc.scalar.copy(ot[:], ps[g][:])
            else:
                nc.vector.tensor_copy(ot[:], ps[g][:])
            nc.sync.dma_start(out=out[g], in_=ot[:])

    nc.compile()
    _STATE["nc"] = nc
    return nc


def _shard(x, W):
    x = np.ascontiguousarray(np.asarray(x, dtype=np.float32))
    W = np.ascontiguousarray(np.asarray(W, dtype=np.float32))
    # xt[t][c, p, g, h, m] = x[2g + m//64, 128t + 64h + (m%64), 128c + p]
    xr = x.reshape(NG, 2, NT, NH, 64, NC_, KC)  # [g, bi, t, h, r, c, p]
    xts = np.ascontiguousarray(np.transpose(xr, (2, 5, 6, 0, 3, 1, 4))).reshape(
        NT, NC_, KC, NG, NH, SB
    )
    # wt[t][c, p, h, n] = W[128c + p, 512t + 256h + n]
    wr = W.reshape(NC_, KC, NT, NH, NW)  # [c, p, t, h, n]
    wts = np.ascontiguousarray(np.transpose(wr, (2, 0, 1, 3, 4)))
    return [{"xt": xts[t], "wt": wts[t]} for t in range(NT)]


def kernel(x, W, _trace=False, _trace_kwargs=None):
    nc = _build()
    in_maps = _shard(x, W)
    res = run_bass_kernel_spmd(
        nc,
        in_maps,
        list(range(NT)),
        trace=_trace,
        **(_trace_kwargs or {}),
    )
    _STATE["last_run"] = res
    # Band extraction: block row m = 64*bi + r holds batch 2g+bi, s-row
    # 128t + 64h + r; band value j sits at block col 256h + 4r + j.
    band = np.empty((B, NT * SB, 4), dtype=np.float32)
    for t in range(NT):
        blk = np.ascontiguousarray(res.results[t]["out"])  # (NG, 128, 512)
        e = blk.strides[2]
        v = np.lib.stride_tricks.as_strided(
            blk,
            shape=(NG, 2, NH, 64, 4),
            strides=(
                blk.strides[0],
                64 * blk.strides[1],
                NW * e,
                blk.strides[1] + 4 * e,
                e,
            ),
        )
        # [g, bi, h, r, j] -> b = 2g + bi, s_rel = 64h + r
        band[:, t * SB : (t + 1) * SB, :] = v.reshape(B, SB, 4)
    s_idx = np.arange(NT * SB)
    y = np.zeros((B, NT * SB, NT * NI), dtype=np.float32)
    y4 = y.reshape(B, NT * SB, NT * SB, 4)
    y4[:, s_idx, s_idx, :] = band
    return y


# revision 8
# speedup vs baseline: 1.9383x; 1.4660x over previous
"""ButterflyLinear Trainium2 kernel.

Math: out[b, s, i] = (sum_o x[b, s, o] * W[o, i]) * mask[s, i], with
mask[s, i] = 1 iff 4s <= i < 4s+4 (stride-4 band). The band makes the
output block-diagonal: s-rows [128t, 128t+128) only touch output columns
[512t, 512t+512) -- an 8x compute reduction vs the full matmul.

Sharding (8 cores): core t owns s-block t for all 16 batches
(tensor-parallel split of W columns; no inter-core communication).

Key packing trick: a 64-row s-sub-block only spans a 256-wide band
window, and that window is the same for every batch. So the matmul
stationary packs TWO batches on the partition axis (M = 128 = 2 batches
x 64 s-rows) against one N=256 W window -- W streams once per batch
PAIR, halving PE row traffic, and each accumulation lives in HALF a
PSUM bank. All 16 chains (8 batch-pairs x 2 sub-blocks) fit in the 8
banks at once: a single wave, no bank-recycling serialization.
PSUM has_written is per-element: only the first matmul of each bank
uses start=True (clears the whole bank), its half-bank partner starts
with start=False and overwrites its untouched half.

Per-core device program:
  - f32 warm-up matmuls during the DMA ramp (HAM clock-gate release)
  - 8 o-chunks streamed (W chunk 256KB + x chunk 1MB each), fp32r
  - 16 chains x 8 chunk-matmuls (N=256) accumulating in half-banks
  - 8 full-bank copies (alternating VectorE/ScalarE) -> DMA raw blocks
Host extracts the 4-wide diagonal band from the raw blocks into the
zero-filled (16, 1024, 4096) result.
"""

import os
import sys
from contextlib import ExitStack

import numpy as np

if "/opt/trn_rl_repo" not in sys.path:
    sys.path.insert(0, "/opt/trn_rl_repo")

import concourse.bass as bass  # noqa: E402
import concourse.tile as tile  # noqa: E402
from concourse import bacc, mybir  # noqa: E402
from concourse.bass_utils import run_bass_kernel_spmd  # noqa: E402

B = 16  # batch
NT = 8  # s-blocks == cores
SB = 128  # s rows per block
NC_ = 8  # o chunks
KC = 128  # o rows per chunk
NI = 512  # output columns per block
NG = 8  # batch pairs
NH = 2  # 64-row s-sub-blocks per s-block
NW = 256  # W window per sub-block
N_WARMUP = int(os.environ.get("BFK_WARMUP", "6"))  # HAM warm-up matmuls

# Matmul input dtype. fp16 (11-bit mantissa) halves DMA traffic and
# streams the PE at 1 cycle/row; measured accuracy is on par with fp32r
# (~1e-4 band rel err) since accumulation stays fp32 in PSUM.
_DT_CHOICES = {
    "f16": mybir.dt.float16,
    "f32r": mybir.dt.float32r,
    "f32": mybir.dt.float32,
}
MM_DT = _DT_CHOICES[os.environ.get("BFK_DT", "f16")]
F32 = mybir.dt.float32

_STATE: dict = {}


def _build():
    if "nc" in _STATE:
        return _STATE["nc"]

    nc = bacc.Bacc(
        "TRN2", target_bir_lowering=False, debug=False, num_devices=NT
    )
    xt = nc.dram_tensor(
        "xt", [NC_, KC, NG, NH, SB], MM_DT, kind="ExternalInput"
    ).ap()
    wt = nc.dram_tensor("wt", [NC_, KC, NH, NW], MM_DT, kind="ExternalInput").ap()
    out = nc.dram_tensor("out", [NG, SB, NI], F32, kind="ExternalOutput").ap()

    with tile.TileContext(nc) as tc, ExitStack() as ctx:
        wp = ctx.enter_context(tc.tile_pool(name="w", bufs=1))
        xp = ctx.enter_context(tc.tile_pool(name="x", bufs=1))
        pp = ctx.enter_context(tc.tile_pool(name="ps", bufs=8, space="PSUM"))
        op = ctx.enter_context(tc.tile_pool(name="o", bufs=6))
        sp = ctx.enter_context(tc.tile_pool(name="scratch", bufs=1))

        # PE warm-up: f32 matmuls (2 HW passes each) on a memset scratch
        # tile, no DMA deps, so they run during the input-DMA ramp and
        # release the HAM clock gate before the real stream starts.
        if N_WARMUP:
            wmt = sp.tile([KC, NI], F32, tag="warm")
            nc.gpsimd.memset(wmt[:], 0.0)
            pwarm = pp.tile([SB, NI], F32, tag="ps", name="ps_warm")
            for _ in range(N_WARMUP):
                nc.tensor.matmul(
                    pwarm[:], wmt[:, :KC], wmt[:], start=True, stop=True
                )

        w_t = []
        x_t = []
        for c in range(NC_):
            w = wp.tile([KC, NH, NW], MM_DT, tag=f"w{c}")
            nc.sync.dma_start(out=w[:], in_=wt[c])
            w_t.append(w)
            xc = xp.tile([KC, NG, NH, SB], MM_DT, tag=f"x{c}")
            nc.sync.dma_start(out=xc[:], in_=xt[c])
            x_t.append(xc)

        ps = [pp.tile([SB, NI], F32, tag="ps", name=f"ps_{g}") for g in range(NG)]
        for c in range(NC_):
            for g in range(NG):
                for h in range(NH):
                    nc.tensor.matmul(
                        ps[g][:, h * NW : (h + 1) * NW],
                        x_t[c][:, g, h, :],
                        w_t[c][:, h, :],
                        start=(c == 0 and h == 0),
                        stop=(c == NC_ - 1 and h == NH - 1),
                    )
        for g in range(NG):
            ot = op.tile([SB, NI], F32, tag="ot")
            # Alternate evacuation between VectorE and ScalarE so two
            # banks drain at a time.
            if g % 2 == 1:
                nc.scalar.copy(ot[:], ps[g][:])
            else:
                nc.vector.tensor_copy(ot[:], ps[g][:])
            nc.sync.dma_start(out=out[g], in_=ot[:])

    nc.compile()
    _STATE["nc"] = nc
    return nc


def _shard(x, W):
    np_dt = mybir.dt.np(MM_DT)
    x = np.ascontiguousarray(np.asarray(x, dtype=np.float32)).astype(np_dt)
    W = np.ascontiguousarray(np.asarray(W, dtype=np.float32)).astype(np_dt)
    # xt[t][c, p, g, h, m] = x[2g + m//64, 128t + 64h + (m%64), 128c + p]
    xr = x.reshape(NG, 2, NT, NH, 64, NC_, KC)  # [g, bi, t, h, r, c, p]
    xts = np.ascontiguousarray(np.transpose(xr, (2, 5, 6, 0, 3, 1, 4))).reshape(
        NT, NC_, KC, NG, NH, SB
    )
    # wt[t][c, p, h, n] = W[128c + p, 512t + 256h + n]
    wr = W.reshape(NC_, KC, NT, NH, NW)  # [c, p, t, h, n]
    wts = np.ascontiguousarray(np.transpose(wr, (2, 0, 1, 3, 4)))
    return [{"xt": xts[t], "wt": wts[t]} for t in range(NT)]


def kernel(x, W, _trace=False, _trace_kwargs=None):
    nc = _build()
    in_maps = _shard(x, W)
    res = run_bass_kernel_spmd(
        nc,
        in_maps,
        list(range(NT)),
        trace=_trace,
        **(_trace_kwargs or {}),
    )
    _STATE["last_run"] = res
    # Band extraction: block row m = 64*bi + r holds batch 2g+bi, s-row
    # 128t + 64h + r; band value j sits at block col 256h + 4r + j.
    band = np.empty((B, NT * SB, 4), dtype=np.float32)
    for t in range(NT):
        blk = np.ascontiguousarray(res.results[t]["out"])  # (NG, 128, 512)
        e = blk.strides[2]
        v = np.lib.stride_tricks.as_strided(
            blk,
            shape=(NG, 2, NH, 64, 4),
            strides=(
                blk.strides[0],
                64 * blk.strides[1],
                NW * e,
                blk.strides[1] + 4 * e,
                e,
            ),
        )
        # [g, bi, h, r, j] -> b = 2g + bi, s_rel = 64h + r
        band[:, t * SB : (t + 1) * SB, :] = v.reshape(B, SB, 4)
    s_idx = np.arange(NT * SB)
    y = np.zeros((B, NT * SB, NT * NI), dtype=np.float32)
    y4 = y.reshape(B, NT * SB, NT * SB, 4)
    y4[:, s_idx, s_idx, :] = band
    return y
